# revision 10
# baseline (speedup 1.0000x reference)
"""Trainium2 Bass kernel for the AttentionLayer problem.

Shapes (hardcoded): B=8, L=1024, D=512, C=4, 8 NeuronCores.
Sharding: data-parallel over batch — core b handles batch b fully
(all 4 channels). No collectives needed.

Per core, for each channel c:
  matl_T = (x @ w_l[c])^T            via PE:  matl_T[e,j] = sum_d w_l[c][d,e] x^T[d,j]
  M      = x @ matl_T                via PE:  M[i,j] = sum_e x^T[e,i] matl_T[e,j]
  s[i]   = sum_j tanh(M[i,j]+b_l[c])  fused: ScalarE tanh with accum_out
  a      = softmax(s + pk)           free-layout softmax on one partition
  sbT    = tanh(w_v2[c]^T-contract x^T + b_v)   [d,j] layout, bias per-partition
  score  = w_v1[c] . sbT + pk        PE matvec, softmax -> abar
  pooled = abar @ x                  PE matvec
  C_l    = a*x + pooled * pad2       rank-1 PE matmul (pad2 x pooled) + DVE fused mul-add
"""

import numpy as np

import concourse.bass as bass
import concourse.mybir as mybir
import concourse.tile as tile
from concourse import bacc
from concourse.bass_utils import run_bass_kernel_spmd
from concourse.masks import make_identity

B, L, D, C = 8, 1024, 512, 4
N_CORES = 8
PT = L // 128  # 8 partition tiles over L
DT = D // 128  # 4 partition tiles over D
JH = 2  # halves of the L free dim (512 each)
F32 = mybir.dt.float32
BF16 = mybir.dt.bfloat16

# dtype for the big matmul chains (matl / M / sb). float32r = 4x faster than
# float32 on the PE at reduced mantissa; producers must round explicitly, so
# all SBUF tensors feeding these matmuls are allocated as MM_DT.
MM_DT = mybir.dt.float32r
ACT = mybir.ActivationFunctionType


def _mm(nc, out, lhsT, rhs, dt, **kw):
    lhsT = lhsT if lhsT.dtype == dt else lhsT.bitcast(dt)
    rhs = rhs if rhs.dtype == dt else rhs.bitcast(dt)
    nc.tensor.matmul(out, lhsT, rhs, **kw)


def build_kernel():
    nc = bacc.Bacc()

    x_ext = nc.declare_dram_parameter("x", [L, D], F32, isOutput=False)
    pad_ext = nc.declare_dram_parameter("pad", [1, L], F32, isOutput=False)
    wl_ext = nc.declare_dram_parameter("w_l", [C, D, D], F32, isOutput=False)
    bl_ext = nc.declare_dram_parameter("b_l", [1, C], F32, isOutput=False)
    wv1_ext = nc.declare_dram_parameter("w_v1", [C, D], F32, isOutput=False)
    wv2_ext = nc.declare_dram_parameter("w_v2", [C, D, D], F32, isOutput=False)
    bv_ext = nc.declare_dram_parameter("b_v", [C, D], F32, isOutput=False)

    feat_ext = nc.declare_dram_parameter("feat", [C, L, D], F32, isOutput=True)
    a_ext = nc.declare_dram_parameter("a_out", [C, L], F32, isOutput=True)
    abar_ext = nc.declare_dram_parameter("abar_out", [C, L], F32, isOutput=True)

    with tile.TileContext(nc) as tc:
        with (
            tc.tile_pool(name="const", bufs=1) as constp,
            tc.tile_pool(name="w", bufs=2) as wp,
            tc.tile_pool(name="matl", bufs=2) as matlp,
            tc.tile_pool(name="sbt", bufs=2) as sbtp,
            tc.tile_pool(name="feat", bufs=4) as featp,
            tc.tile_pool(name="sm", bufs=2) as smp,
            tc.tile_pool(name="psM", bufs=2, space="PSUM") as psM,
            tc.tile_pool(name="psml", bufs=2, space="PSUM") as psml,
            tc.tile_pool(name="pss", bufs=2, space="PSUM") as pss,
        ):
            # ---- constants / preload ----
            ident = constp.tile([128, 128], F32)
            make_identity(nc, ident[:])
            ones_row = constp.tile([1, 128], F32)
            nc.vector.memset(ones_row[:], 1.0)

            # x tiled [128, PT, D]
            x_sb = constp.tile([128, PT, D], F32)
            nc.sync.dma_start(
                x_sb[:], x_ext[:].rearrange("(t p) d -> p t d", p=128)
            )
            # pad row + pad2 row
            pad_free = constp.tile([1, L], F32)
            nc.sync.dma_start(pad_free[:], pad_ext[:])
            pad2_free = constp.tile([1, L], F32)
            nc.scalar.activation(
                pad2_free[:], pad_free[:], ACT.Identity, bias=1.0, scale=1.0 / 99999.0
            )
            # pk in column layout [128, PT]
            pk_col = constp.tile([128, PT], F32)
            nc.sync.dma_start(
                pk_col[:], pad_ext[:].rearrange("o (t p) -> (o p) t", p=128)
            )
            # b_l broadcast to all partitions [128, C]
            bl_row = constp.tile([1, C], F32)
            nc.sync.dma_start(bl_row[:], bl_ext[:])
            bl_bc = constp.tile([128, C], F32)
            nc.gpsimd.partition_broadcast(bl_bc[:], bl_row[:])
            # w_v1 as [128, C*DT] columns (col = c*DT + k), bf16 for score matmul
            wv1_col = constp.tile([128, C * DT], F32)
            nc.sync.dma_start(
                wv1_col[:], wv1_ext[:].rearrange("c (k p) -> p (c k)", p=128)
            )
            wv1_bf = constp.tile([128, C * DT], BF16)
            nc.vector.tensor_copy(wv1_bf[:], wv1_col[:])
            # b_v as [128, C*DT] columns (bias per d-partition)
            bv_col = constp.tile([128, C * DT], F32)
            nc.sync.dma_start(
                bv_col[:], bv_ext[:].rearrange("c (k p) -> p (c k)", p=128)
            )

            # bf16 copy of x for the pooled matvec
            x_bf = constp.tile([128, PT, D], BF16)
            for t in range(PT):
                nc.vector.tensor_copy(x_bf[:, t, :], x_sb[:, t, :])
            # bf16 pad2 row for the rank-1 poolpad matmul
            pad2_bf = constp.tile([1, L], BF16)
            nc.vector.tensor_copy(pad2_bf[:], pad2_free[:])

            # xT [128, DT, L] via PE transposes
            xT_sb = constp.tile([128, DT, L], MM_DT)
            for dt_i in range(DT):
                for lt in range(PT):
                    pst = pss.tile([128, 128], F32, tag="ps_small")
                    nc.tensor.transpose(
                        pst[:],
                        x_sb[:, lt, dt_i * 128 : (dt_i + 1) * 128],
                        ident[:],
                    )
                    nc.vector.tensor_copy(
                        xT_sb[:, dt_i, lt * 128 : (lt + 1) * 128], pst[:]
                    )

            # per-channel weights, emitted up front (pool bufs pipeline them)
            wl_sb = [
                wp.tile([128, DT, D], MM_DT, tag="wl", name=f"wl{c}")
                for c in range(C)
            ]
            wv2_sb = [
                wp.tile([128, DT, D], MM_DT, tag="wv2", name=f"wv2_{c}")
                for c in range(C)
            ]
            for c in range(C):
                wl_stg = wp.tile([128, DT, D], F32, tag="wstg", name=f"wls{c}")
                nc.sync.dma_start(
                    wl_stg[:], wl_ext[c].rearrange("(k p) e -> p k e", p=128)
                )
                nc.vector.tensor_copy(wl_sb[c][:], wl_stg[:])
                wv2_stg = wp.tile([128, DT, D], F32, tag="wstg", name=f"wvs{c}")
                nc.sync.dma_start(
                    wv2_stg[:], wv2_ext[c].rearrange("(k p) e -> p k e", p=128)
                )
                nc.vector.tensor_copy(wv2_sb[c][:], wv2_stg[:])

            for c in range(C):
                # ---- A: matl_T[c] [D, L] = (x @ w_l[c])^T ----
                matl = matlp.tile([128, DT, L], MM_DT)
                for et in range(DT):
                    for jh in range(JH):
                        pml = psml.tile([128, 512], F32, tag="psml")
                        for k in range(DT):
                            _mm(
                                nc,
                                pml[:],
                                wl_sb[c][:, k, et * 128 : (et + 1) * 128],
                                xT_sb[:, k, jh * 512 : (jh + 1) * 512],
                                MM_DT,
                                start=(k == 0),
                                stop=(k == DT - 1),
                            )
                        nc.vector.tensor_copy(
                            matl[:, et, jh * 512 : (jh + 1) * 512], pml[:]
                        )

                # ---- B: M tiles + fused tanh/rowsum ----
                s_tile = smp.tile([128, PT], F32, tag="s")
                for it in range(PT):
                    pM = psM.tile([128, L], F32)
                    for jh in range(JH):
                        for k in range(DT):
                            _mm(
                                nc,
                                pM[:, jh * 512 : (jh + 1) * 512],
                                xT_sb[:, k, it * 128 : (it + 1) * 128],
                                matl[:, k, jh * 512 : (jh + 1) * 512],
                                MM_DT,
                                start=(k == 0),
                                stop=(k == DT - 1),
                            )
                    nc.scalar.activation(
                        pM[:],
                        pM[:],
                        ACT.Tanh,
                        bias=bl_bc[:, c : c + 1],
                        accum_out=s_tile[:, it : it + 1],
                    )

                # ---- C: softmax over i (partition axis) for `a` ----
                nc.vector.tensor_add(s_tile[:], s_tile[:], pk_col[:])
                psT = pss.tile([PT, 128], F32, tag="ps_small")
                nc.tensor.transpose(psT[:], s_tile[:], ident[:])
                sT8 = smp.tile([PT, 128], F32, tag="sT8")
                nc.vector.tensor_copy(sT8[:], psT[:])
                s_free = smp.tile([1, L], F32, tag="sfree")
                nc.sync.dma_start(s_free[:], sT8[:])
                mx = smp.tile([1, 4], F32, tag="mx")
                nc.vector.tensor_reduce(
                    mx[:, 0:1], s_free[:], mybir.AxisListType.X, mybir.AluOpType.max
                )
                nc.vector.tensor_scalar_mul(mx[:, 1:2], mx[:, 0:1], -1.0)
                e_free = smp.tile([1, L], F32, tag="efree")
                nc.scalar.activation(e_free[:], s_free[:], ACT.Exp, bias=mx[:, 1:2])
                nc.vector.tensor_reduce(
                    mx[:, 2:3], e_free[:], mybir.AxisListType.X, mybir.AluOpType.add
                )
                nc.vector.reciprocal(mx[:, 3:4], mx[:, 2:3])
                a_free = smp.tile([1, L], F32, tag="afree")
                nc.vector.tensor_scalar_mul(a_free[:], e_free[:], mx[:, 3:4])
                nc.sync.dma_start(a_ext[c : c + 1, :], a_free[:])
                # broadcast (-max, 1/sum) to all partitions via ones-matmul
                pair = smp.tile([1, 2], F32, tag="pair")
                nc.vector.tensor_copy(pair[:, 0:1], mx[:, 1:2])
                nc.vector.tensor_copy(pair[:, 1:2], mx[:, 3:4])
                pbc = pss.tile([128, 2], F32, tag="ps_small")
                _mm(nc, pbc[:], ones_row[:], pair[:], F32, start=True, stop=True)
                bc_sb = smp.tile([128, 2], F32, tag="bc")
                nc.vector.tensor_copy(bc_sb[:], pbc[:])
                e_col = smp.tile([128, PT], F32, tag="ecol")
                nc.scalar.activation(e_col[:], s_tile[:], ACT.Exp, bias=bc_sb[:, 0:1])
                a_col = smp.tile([128, PT], F32, tag="acol")
                nc.vector.tensor_scalar_mul(a_col[:], e_col[:], bc_sb[:, 1:2])

                # ---- D: sbT[c] [D, L] = tanh(w_v2-contract + b_v), bf16 ----
                sbt = sbtp.tile([128, DT, L], BF16)
                for dt_i in range(DT):
                    for jh in range(JH):
                        pv = psml.tile([128, 512], F32, tag="psml")
                        for k in range(DT):
                            _mm(
                                nc,
                                pv[:],
                                wv2_sb[c][:, k, dt_i * 128 : (dt_i + 1) * 128],
                                xT_sb[:, k, jh * 512 : (jh + 1) * 512],
                                MM_DT,
                                start=(k == 0),
                                stop=(k == DT - 1),
                            )
                        nc.scalar.activation(
                            sbt[:, dt_i, jh * 512 : (jh + 1) * 512],
                            pv[:],
                            ACT.Tanh,
                            bias=bv_col[:, c * DT + dt_i : c * DT + dt_i + 1],
                        )

                # ---- E: score = w_v1 . sbT + pk  (free layout [1, L]) ----
                score = smp.tile([1, L], F32, tag="score")
                for jh in range(JH):
                    psc = pss.tile([1, 512], F32, tag="ps_small")
                    for k in range(DT):
                        nc.tensor.matmul(
                            psc[:],
                            wv1_bf[:, c * DT + k : c * DT + k + 1],
                            sbt[:, k, jh * 512 : (jh + 1) * 512],
                            start=(k == 0),
                            stop=(k == DT - 1),
                        )
                    nc.vector.scalar_tensor_tensor(
                        score[:, jh * 512 : (jh + 1) * 512],
                        psc[:],
                        1.0,
                        pad_free[:, jh * 512 : (jh + 1) * 512],
                        op0=mybir.AluOpType.mult,
                        op1=mybir.AluOpType.add,
                    )

                # ---- F: softmax -> abar (free layout), then to column layout ----
                mx2 = smp.tile([1, 4], F32, tag="mx2")
                nc.vector.tensor_reduce(
                    mx2[:, 0:1], score[:], mybir.AxisListType.X, mybir.AluOpType.max
                )
                nc.vector.tensor_scalar_mul(mx2[:, 1:2], mx2[:, 0:1], -1.0)
                eb_free = smp.tile([1, L], F32, tag="ebfree")
                nc.scalar.activation(eb_free[:], score[:], ACT.Exp, bias=mx2[:, 1:2])
                nc.vector.tensor_reduce(
                    mx2[:, 2:3], eb_free[:], mybir.AxisListType.X, mybir.AluOpType.add
                )
                nc.vector.reciprocal(mx2[:, 3:4], mx2[:, 2:3])
                abar_free = smp.tile([1, L], F32, tag="abfree")
                nc.vector.tensor_scalar_mul(abar_free[:], eb_free[:], mx2[:, 3:4])
                nc.sync.dma_start(abar_ext[c : c + 1, :], abar_free[:])
                ab8 = smp.tile([PT, 128], F32, tag="ab8")
                nc.sync.dma_start(ab8[:], abar_free[:])
                pab = pss.tile([128, PT], F32, tag="ps_small")
                nc.tensor.transpose(pab[:], ab8[:], ident[0:PT, 0:PT])
                abar_col = smp.tile([128, PT], BF16, tag="abcol")
                nc.vector.tensor_copy(abar_col[:], pab[:])

                # ---- G: pooled = abar @ x  [1, D] ----
                ppl = pss.tile([1, D], F32, tag="ps_small")
                for t in range(PT):
                    nc.tensor.matmul(
                        ppl[:],
                        abar_col[:, t : t + 1],
                        x_bf[:, t, :],
                        start=(t == 0),
                        stop=(t == PT - 1),
                    )
                pooled = smp.tile([1, D], BF16, tag="pooled")
                nc.vector.tensor_copy(pooled[:], ppl[:])

                # ---- H: epilogue C_l = a*x + (pad2 x pooled) ----
                for it in range(PT):
                    ppp = pss.tile([128, D], F32, tag="ps_small")
                    nc.tensor.matmul(
                        ppp[:],
                        pad2_bf[:, it * 128 : (it + 1) * 128],
                        pooled[:],
                        start=True,
                        stop=True,
                    )
                    ft = featp.tile([128, D], F32)
                    nc.vector.scalar_tensor_tensor(
                        ft[:],
                        x_sb[:, it, :],
                        a_col[:, it : it + 1],
                        ppp[:],
                        op0=mybir.AluOpType.mult,
                        op1=mybir.AluOpType.add,
                    )
                    nc.sync.dma_start(
                        feat_ext[c, it * 128 : (it + 1) * 128, :], ft[:]
                    )

    nc.compile()
    return nc


_NC_CACHE = None
_LAST_RES = None


def kernel(inputs, pad_k, w_l, b_l, w_v1, w_v2, b_v):
    global _NC_CACHE
    inputs = np.ascontiguousarray(np.asarray(inputs, dtype=np.float32))
    pad_k = np.ascontiguousarray(np.asarray(pad_k, dtype=np.float32))
    w_l = np.ascontiguousarray(np.asarray(w_l, dtype=np.float32))
    b_l = np.ascontiguousarray(np.asarray(b_l, dtype=np.float32))
    w_v1 = np.ascontiguousarray(np.asarray(w_v1, dtype=np.float32))
    w_v2 = np.ascontiguousarray(np.asarray(w_v2, dtype=np.float32))
    b_v = np.ascontiguousarray(np.asarray(b_v, dtype=np.float32))

    if _NC_CACHE is None:
        _NC_CACHE = build_kernel()
    nc = _NC_CACHE

    in_maps = []
    for b in range(N_CORES):
        in_maps.append(
            {
                "x": inputs[b],
                "pad": pad_k[b].reshape(1, L),
                "w_l": w_l,
                "b_l": b_l.reshape(1, C),
                "w_v1": w_v1,
                "w_v2": w_v2,
                "b_v": b_v,
            }
        )
    res = run_bass_kernel_spmd(nc, in_maps, core_ids=list(range(N_CORES)))
    global _LAST_RES
    _LAST_RES = res

    C_features = np.empty((B, L, D, C + 1), dtype=np.float32)
    a = np.empty((B, C, L, 1), dtype=np.float32)
    abar = np.empty((B, C, L, 1), dtype=np.float32)
    for b in range(N_CORES):
        r = res.results[b]
        C_features[b, :, :, :C] = r["feat"].transpose(1, 2, 0)
        C_features[b, :, :, C] = inputs[b]
        a[b] = r["a_out"][:, :, None]
        abar[b] = r["abar_out"][:, :, None]
    return C_features, a, abar
